# revision 2
# baseline (speedup 1.0000x reference)
"""Bilateral filter denoiser (5x5, sigma_s=2.0, sigma_r=0.1) on 8 Trainium2
NeuronCores.  Takes full inputs x (8,3,512,512) f32 + spatial (5,5) f32;
pure data parallel: one batch element per core; returns the full output.

Per-core kernel (Bass/Tile), symmetric half-offset formulation:
  For each of the 12 half offsets t=(di,dj) the range/spatial weight field
      W_t[g] = exp(-50*(xp[g+t]-xp[g])^2 + ln s_t)
  is shared by the forward tap (pixel g gathering from g+t) and the backward
  tap (pixel g+t gathering from g).  With m_t = W_t*(xp[g+t]-xp[g]):
      S0 = s_c + sum_t (W_t[g] + W_t[g-t]),  S1 = sum_t (m_t[g] - m_t[g-t])
      out = x + S1/S0        (S0 >= s_c > 0: the 1e-10 clip never binds)
  Reflect padding makes the shared-weight trick exact at image borders.

Implementation notes:
  * Channels are flattened along rows: xp [Rpad, 516] fp16 (host-padded).
    13 strips of 128 consecutive padded rows; 3 row-shifted slab copies
    T0..T2 are DMA-loaded so every compute operand starts at partition 0
    (HW constraint: compute start-partition must be 0/32/64/96).
  * Fields d/sq/W/m are fp16; Square runs 6 offsets on ScalarE, 6 on
    VectorE (engine balance); Exp folds the -50 scale and ln(spatial) bias.
  * The 4 shifted accumulations per offset run on the (otherwise idle)
    TensorE as matmuls with host-provided +-shifted-identity matrices,
    accumulating S0/S1 in PSUM in fp32.  s_center enters via s_c*I @ ones.
  * Epilogue: custom-DVE fast reciprocal, fused on VectorE; result fp32.
"""

import numpy as np

B, C, H, W = 8, 3, 512, 512
SIGMA_R = 0.1
EXP_SCALE = -1.0 / (2.0 * SIGMA_R * SIGMA_R)  # -50

HALF_OFFSETS = [
    (0, 1), (0, 2),
    (1, -2), (1, -1), (1, 0), (1, 1), (1, 2),
    (2, -2), (2, -1), (2, 0), (2, 1), (2, 2),
]
SQ_ON_ACT = {(0, 2), (1, -2), (1, 0), (1, 2), (2, -1), (2, 1)}

_CACHE = {}


def _strip_plan():
    Hp = H + 4
    R = C * Hp
    strips = []
    rbase = 0
    while R - 4 - rbase > 0:
        strips.append((rbase, min(124, R - 4 - rbase)))
        rbase += 124
    return strips[-1][0] + 132, strips


def _build():
    from contextlib import ExitStack

    import concourse.bacc as bacc
    import concourse.tile as tile
    from concourse import mybir

    F32 = mybir.dt.float32
    F16 = mybir.dt.float16
    Alu = mybir.AluOpType
    Act = mybir.ActivationFunctionType

    Hp, Wp = H + 4, W + 4
    R = C * Hp
    Rpad, strips = _strip_plan()

    nc = bacc.Bacc(
        "TRN2",
        target_bir_lowering=False,
        debug=False,
        enable_asserts=False,
        num_devices=B,
    )
    xp = nc.dram_tensor("xp", [Rpad, Wp], F16, kind="ExternalInput").ap()
    lsp = nc.dram_tensor("lsp", [1, 25], F32, kind="ExternalInput").ap()
    shm = nc.dram_tensor("shm", [7 * 128, 128], F16, kind="ExternalInput").ap()
    y = nc.dram_tensor("y", [C * H, W], F32, kind="ExternalOutput").ap()

    with tile.TileContext(nc) as tc, ExitStack() as ctx:
        consts = ctx.enter_context(tc.tile_pool(name="consts", bufs=1))
        lt = consts.tile([128, 25], F32)
        nc.gpsimd.dma_start(out=lt[:], in_=lsp.to_broadcast([128, 25]))
        M = []  # M[0..2] = eye(k=-s), M[3..5] = -eye(k=-s), M[6] = s_c*I
        for s in range(7):
            m_ = consts.tile([128, 128], F16, name=f"shm{s}", tag=f"shm{s}")
            nc.sync.dma_start(out=m_[:, :], in_=shm[s * 128:(s + 1) * 128, :])
            M.append(m_)
        ones = consts.tile([128, W], F16)
        nc.vector.memset(ones[:], 1.0)

        slabs = ctx.enter_context(tc.tile_pool(name="slabs", bufs=2))
        fld = ctx.enter_context(tc.tile_pool(name="fld", bufs=8))
        accp = ctx.enter_context(tc.tile_pool(name="accum", bufs=2))
        psum = ctx.enter_context(tc.tile_pool(name="psum", bufs=4, space="PSUM"))

        for rbase, K in strips:
            T = [slabs.tile([128, Wp], F16, tag=f"T{v}", name=f"T{v}")
                 for v in range(3)]
            for v in range(3):
                nc.sync.dma_start(out=T[v][:, :],
                                  in_=xp[rbase + v:rbase + v + 128, :])

            S0p = psum.tile([128, W], F32, tag="S0p", name="S0p")
            S1p = psum.tile([128, W], F32, tag="S1p", name="S1p")
            # s_center into S0 (SC @ ones); starts the S0 accumulation group
            nc.tensor.matmul(S0p[:, :], M[6][:, :], ones[:, :],
                             start=True, stop=False)

            for oi, (di, dj) in enumerate(HALF_OFFSETS):
                cl = min(0, -dj)
                ch = max(W, W - dj)
                wd = ch - cl
                t_bias = (di + 2) * 5 + (dj + 2)

                dt = fld.tile([128, Wp], F16, tag="dt", name="dt")
                nc.vector.tensor_tensor(
                    out=dt[:, :wd],
                    in0=T[di][:, cl + dj + 2:ch + dj + 2],
                    in1=T[0][:, cl + 2:ch + 2],
                    op=Alu.subtract)
                sq = fld.tile([128, Wp], F16, tag="sq", name="sq")
                if (di, dj) in SQ_ON_ACT:
                    nc.scalar.activation(sq[:, :wd], dt[:, :wd], Act.Square)
                else:
                    nc.vector.tensor_tensor(
                        out=sq[:, :wd], in0=dt[:, :wd], in1=dt[:, :wd],
                        op=Alu.mult)
                Wt = fld.tile([128, Wp], F16, tag="Wt", name="Wt")
                nc.scalar.activation(
                    Wt[:, :wd], sq[:, :wd], Act.Exp,
                    bias=lt[:, t_bias:t_bias + 1], scale=EXP_SCALE)
                mt = fld.tile([128, Wp], F16, tag="mt", name="mt")
                nc.vector.tensor_tensor(
                    out=mt[:, :wd], in0=Wt[:, :wd], in1=dt[:, :wd],
                    op=Alu.mult)

                fw = -cl          # field col of output j=0, forward view
                bw = -dj - cl     # field col of output j=0, backward view
                last = oi == len(HALF_OFFSETS) - 1
                nc.tensor.matmul(S0p[:, :], M[2][:, :], Wt[:, fw:fw + W],
                                 start=False, stop=False)
                nc.tensor.matmul(S0p[:, :], M[2 - di][:, :], Wt[:, bw:bw + W],
                                 start=False, stop=last)
                nc.tensor.matmul(S1p[:, :], M[2][:, :], mt[:, fw:fw + W],
                                 start=(oi == 0), stop=False)
                nc.tensor.matmul(S1p[:, :], M[3 + 2 - di][:, :],
                                 mt[:, bw:bw + W], start=False, stop=last)

            Rc = accp.tile([128, W], F32, tag="Rc", name="Rc")
            nc.vector.reciprocal_approx_fast(out=Rc[:K, :], in_=S0p[:K, :])
            tmp = accp.tile([128, W], F32, tag="tmp", name="tmp")
            nc.vector.tensor_tensor(
                out=tmp[:K, :], in0=S1p[:K, :], in1=Rc[:K, :], op=Alu.mult)
            res = accp.tile([128, W], F32, tag="res", name="res")
            nc.vector.tensor_tensor(
                out=res[:K, :], in0=tmp[:K, :], in1=T[2][0:K, 2:2 + W],
                op=Alu.add)

            # DMA out valid rows (skip per-channel halo rows)
            k = 0
            while k < K:
                g = rbase + 2 + k
                if g < R and 2 <= (g % Hp) <= Hp - 3:
                    k1 = k
                    while k1 < K:
                        g1 = rbase + 2 + k1
                        if g1 >= R or not (2 <= (g1 % Hp) <= Hp - 3):
                            break
                        if (g1 % Hp) == 2 and k1 > k:
                            break
                        k1 += 1
                    h0 = (g // Hp) * H + (g % Hp) - 2
                    nc.sync.dma_start(out=y[h0:h0 + (k1 - k), :],
                                      in_=res[k:k1, :])
                    k = k1
                else:
                    k += 1

    nc.compile()
    return nc


def _get_module():
    if "nc" not in _CACHE:
        _CACHE["nc"] = _build()
    return _CACHE["nc"]


def _pack_core(xc):
    """xc [C,H,W] f32 -> reflect-padded fp16 [Rpad, W+4]."""
    Rpad, _ = _strip_plan()
    xpad = np.pad(xc, ((0, 0), (2, 2), (2, 2)), mode="reflect")
    flat = xpad.reshape(C * (H + 4), W + 4)
    extra = Rpad - flat.shape[0]
    if extra > 0:
        flat = np.concatenate([flat, np.repeat(flat[-1:], extra, axis=0)],
                              axis=0)
    return np.ascontiguousarray(flat, dtype=np.float16)


def kernel(x, spatial, _trace=False):
    from concourse.bass_utils import run_bass_kernel_spmd

    x = np.asarray(x, dtype=np.float32)
    spatial = np.asarray(spatial, dtype=np.float32)
    assert x.shape == (B, C, H, W) and spatial.shape == (5, 5)
    # the kernel shares each half-offset's weight field between its forward
    # and backward taps, which requires a symmetric spatial kernel
    assert np.allclose(spatial, spatial[::-1, ::-1], rtol=1e-5), \
        "kernel assumes point-symmetric spatial weights"

    lsp = np.log(spatial).reshape(1, 25).astype(np.float32)
    mats = [np.eye(128, 128, k=-s, dtype=np.float16) for s in range(3)]
    mats += [-np.eye(128, 128, k=-s, dtype=np.float16) for s in range(3)]
    mats.append(np.eye(128, dtype=np.float16)
                * np.float16(float(spatial[2, 2])))
    shm = np.concatenate(mats, axis=0)

    nc = _get_module()
    in_maps = [{"xp": _pack_core(x[b]), "lsp": lsp, "shm": shm}
               for b in range(B)]
    res = run_bass_kernel_spmd(nc, in_maps, core_ids=list(range(B)),
                               trace=_trace)
    out = np.stack([res.results[b]["y"].reshape(C, H, W) for b in range(B)])
    if _trace:
        return out.astype(np.float32), res
    return out.astype(np.float32)


# revision 3
# speedup vs baseline: 1.0060x; 1.0060x over previous
"""Bilateral filter denoiser (5x5, sigma_s=2.0, sigma_r=0.1) on 8 Trainium2
NeuronCores.  Takes full inputs x (8,3,512,512) f32 + spatial (5,5) f32;
pure data parallel: one batch element per core; returns the full output.

Per-core kernel (Bass/Tile), symmetric half-offset formulation:
  For each of the 12 half offsets t=(di,dj) the range/spatial weight field
      W_t[g] = exp(-50*(xp[g+t]-xp[g])^2 + ln s_t)
  is shared by the forward tap (pixel g gathering from g+t) and the backward
  tap (pixel g+t gathering from g).  With m_t = W_t*(xp[g+t]-xp[g]):
      S0 = s_c + sum_t (W_t[g] + W_t[g-t]),  S1 = sum_t (m_t[g] - m_t[g-t])
      out = x + S1/S0        (S0 >= s_c > 0: the 1e-10 clip never binds)
  Reflect padding makes the shared-weight trick exact at image borders.

Implementation notes:
  * Channels are flattened along rows: xp [Rpad, 516] fp16 (host-padded).
    13 strips of 128 consecutive padded rows; 3 row-shifted slab copies
    T0..T2 are DMA-loaded so every compute operand starts at partition 0
    (HW constraint: compute start-partition must be 0/32/64/96).
  * Fields d/sq/W/m are fp16; Square runs 6 offsets on ScalarE, 6 on
    VectorE (engine balance); Exp folds the -50 scale and ln(spatial) bias.
  * The 4 shifted accumulations per offset run on the (otherwise idle)
    TensorE as matmuls with host-provided +-shifted-identity matrices,
    accumulating S0/S1 in PSUM in fp32.  s_center enters via s_c*I @ ones.
  * Epilogue: custom-DVE fast reciprocal, fused on VectorE; result fp32.
"""

import numpy as np

B, C, H, W = 8, 3, 512, 512
SIGMA_R = 0.1
EXP_SCALE = -1.0 / (2.0 * SIGMA_R * SIGMA_R)  # -50

HALF_OFFSETS = [
    (0, 1), (0, 2),
    (1, -2), (1, -1), (1, 0), (1, 1), (1, 2),
    (2, -2), (2, -1), (2, 0), (2, 1), (2, 2),
]
SQ_ON_ACT = {(0, 2), (1, -2), (1, 0), (1, 2), (2, -1), (2, 1)}

_CACHE = {}


def _strip_plan():
    Hp = H + 4
    R = C * Hp
    strips = []
    rbase = 0
    while R - 4 - rbase > 0:
        strips.append((rbase, min(124, R - 4 - rbase)))
        rbase += 124
    return strips[-1][0] + 132, strips


def _build():
    from contextlib import ExitStack

    import concourse.bacc as bacc
    import concourse.tile as tile
    from concourse import mybir

    F32 = mybir.dt.float32
    F16 = mybir.dt.float16
    Alu = mybir.AluOpType
    Act = mybir.ActivationFunctionType

    Hp, Wp = H + 4, W + 4
    R = C * Hp
    Rpad, strips = _strip_plan()

    nc = bacc.Bacc(
        "TRN2",
        target_bir_lowering=False,
        debug=False,
        enable_asserts=False,
        num_devices=B,
    )
    xp = nc.dram_tensor("xp", [Rpad, Wp], F16, kind="ExternalInput").ap()
    lsp = nc.dram_tensor("lsp", [1, 25], F32, kind="ExternalInput").ap()
    shm = nc.dram_tensor("shm", [7 * 128, 128], F16, kind="ExternalInput").ap()
    y = nc.dram_tensor("y", [C * H, W], F32, kind="ExternalOutput").ap()

    with tile.TileContext(nc) as tc, ExitStack() as ctx:
        consts = ctx.enter_context(tc.tile_pool(name="consts", bufs=1))
        lt = consts.tile([128, 25], F32)
        nc.gpsimd.dma_start(out=lt[:], in_=lsp.to_broadcast([128, 25]))
        M = []  # M[0..2] = eye(k=-s), M[3..5] = -eye(k=-s), M[6] = s_c*I
        for s in range(7):
            m_ = consts.tile([128, 128], F16, name=f"shm{s}", tag=f"shm{s}")
            nc.sync.dma_start(out=m_[:, :], in_=shm[s * 128:(s + 1) * 128, :])
            M.append(m_)
        ones = consts.tile([128, W], F16)
        nc.vector.memset(ones[:], 1.0)

        slabs = ctx.enter_context(tc.tile_pool(name="slabs", bufs=2))
        fld = ctx.enter_context(tc.tile_pool(name="fld", bufs=12))
        accp = ctx.enter_context(tc.tile_pool(name="accum", bufs=2))
        psum = ctx.enter_context(tc.tile_pool(name="psum", bufs=4, space="PSUM"))

        for rbase, K in strips:
            T = [slabs.tile([128, Wp], F16, tag=f"T{v}", name=f"T{v}")
                 for v in range(3)]
            for v in range(3):
                nc.sync.dma_start(out=T[v][:, :],
                                  in_=xp[rbase + v:rbase + v + 128, :])

            S0p = psum.tile([128, W], F32, tag="S0p", name="S0p")
            S1p = psum.tile([128, W], F32, tag="S1p", name="S1p")
            # s_center into S0 (SC @ ones); starts the S0 accumulation group
            nc.tensor.matmul(S0p[:, :], M[6][:, :], ones[:, :],
                             start=True, stop=False)

            for oi, (di, dj) in enumerate(HALF_OFFSETS):
                cl = min(0, -dj)
                ch = max(W, W - dj)
                wd = ch - cl
                t_bias = (di + 2) * 5 + (dj + 2)

                dt = fld.tile([128, Wp], F16, tag="dt", name="dt")
                nc.vector.tensor_tensor(
                    out=dt[:, :wd],
                    in0=T[di][:, cl + dj + 2:ch + dj + 2],
                    in1=T[0][:, cl + 2:ch + 2],
                    op=Alu.subtract)
                sq = fld.tile([128, Wp], F16, tag="sq", name="sq")
                if (di, dj) in SQ_ON_ACT:
                    nc.scalar.activation(sq[:, :wd], dt[:, :wd], Act.Square)
                else:
                    nc.vector.tensor_tensor(
                        out=sq[:, :wd], in0=dt[:, :wd], in1=dt[:, :wd],
                        op=Alu.mult)
                Wt = fld.tile([128, Wp], F16, tag="Wt", name="Wt")
                nc.scalar.activation(
                    Wt[:, :wd], sq[:, :wd], Act.Exp,
                    bias=lt[:, t_bias:t_bias + 1], scale=EXP_SCALE)
                mt = fld.tile([128, Wp], F16, tag="mt", name="mt")
                nc.vector.tensor_tensor(
                    out=mt[:, :wd], in0=Wt[:, :wd], in1=dt[:, :wd],
                    op=Alu.mult)

                fw = -cl          # field col of output j=0, forward view
                bw = -dj - cl     # field col of output j=0, backward view
                last = oi == len(HALF_OFFSETS) - 1
                nc.tensor.matmul(S0p[:, :], M[2][:, :], Wt[:, fw:fw + W],
                                 start=False, stop=False)
                nc.tensor.matmul(S0p[:, :], M[2 - di][:, :], Wt[:, bw:bw + W],
                                 start=False, stop=last)
                nc.tensor.matmul(S1p[:, :], M[2][:, :], mt[:, fw:fw + W],
                                 start=(oi == 0), stop=False)
                nc.tensor.matmul(S1p[:, :], M[3 + 2 - di][:, :],
                                 mt[:, bw:bw + W], start=False, stop=last)

            Rc = accp.tile([128, W], F32, tag="Rc", name="Rc")
            nc.vector.reciprocal_approx_fast(out=Rc[:K, :], in_=S0p[:K, :])
            tmp = accp.tile([128, W], F32, tag="tmp", name="tmp")
            nc.vector.tensor_tensor(
                out=tmp[:K, :], in0=S1p[:K, :], in1=Rc[:K, :], op=Alu.mult)
            res = accp.tile([128, W], F32, tag="res", name="res")
            nc.vector.tensor_tensor(
                out=res[:K, :], in0=tmp[:K, :], in1=T[2][0:K, 2:2 + W],
                op=Alu.add)

            # DMA out valid rows (skip per-channel halo rows)
            k = 0
            while k < K:
                g = rbase + 2 + k
                if g < R and 2 <= (g % Hp) <= Hp - 3:
                    k1 = k
                    while k1 < K:
                        g1 = rbase + 2 + k1
                        if g1 >= R or not (2 <= (g1 % Hp) <= Hp - 3):
                            break
                        if (g1 % Hp) == 2 and k1 > k:
                            break
                        k1 += 1
                    h0 = (g // Hp) * H + (g % Hp) - 2
                    nc.sync.dma_start(out=y[h0:h0 + (k1 - k), :],
                                      in_=res[k:k1, :])
                    k = k1
                else:
                    k += 1

    nc.compile()
    return nc


def _get_module():
    if "nc" not in _CACHE:
        _CACHE["nc"] = _build()
    return _CACHE["nc"]


def _pack_core(xc):
    """xc [C,H,W] f32 -> reflect-padded fp16 [Rpad, W+4]."""
    Rpad, _ = _strip_plan()
    xpad = np.pad(xc, ((0, 0), (2, 2), (2, 2)), mode="reflect")
    flat = xpad.reshape(C * (H + 4), W + 4)
    extra = Rpad - flat.shape[0]
    if extra > 0:
        flat = np.concatenate([flat, np.repeat(flat[-1:], extra, axis=0)],
                              axis=0)
    return np.ascontiguousarray(flat, dtype=np.float16)


def kernel(x, spatial, _trace=False):
    from concourse.bass_utils import run_bass_kernel_spmd

    x = np.asarray(x, dtype=np.float32)
    spatial = np.asarray(spatial, dtype=np.float32)
    assert x.shape == (B, C, H, W) and spatial.shape == (5, 5)
    # the kernel shares each half-offset's weight field between its forward
    # and backward taps, which requires a symmetric spatial kernel
    assert np.allclose(spatial, spatial[::-1, ::-1], rtol=1e-5), \
        "kernel assumes point-symmetric spatial weights"

    lsp = np.log(spatial).reshape(1, 25).astype(np.float32)
    mats = [np.eye(128, 128, k=-s, dtype=np.float16) for s in range(3)]
    mats += [-np.eye(128, 128, k=-s, dtype=np.float16) for s in range(3)]
    mats.append(np.eye(128, dtype=np.float16)
                * np.float16(float(spatial[2, 2])))
    shm = np.concatenate(mats, axis=0)

    nc = _get_module()
    in_maps = [{"xp": _pack_core(x[b]), "lsp": lsp, "shm": shm}
               for b in range(B)]
    res = run_bass_kernel_spmd(nc, in_maps, core_ids=list(range(B)),
                               trace=_trace)
    out = np.stack([res.results[b]["y"].reshape(C, H, W) for b in range(B)])
    if _trace:
        return out.astype(np.float32), res
    return out.astype(np.float32)


# revision 4
# speedup vs baseline: 1.0682x; 1.0619x over previous
"""Bilateral filter denoiser (5x5, sigma_s=2.0, sigma_r=0.1) on 8 Trainium2
NeuronCores.  Takes full inputs x (8,3,512,512) f32 + spatial (5,5) f32;
pure data parallel: one batch element per core; returns the full output.

Per-core kernel (Bass/Tile), symmetric half-offset formulation:
  For each of the 12 half offsets t=(di,dj) the range/spatial weight field
      W_t[g] = exp(-50*(xp[g+t]-xp[g])^2 + ln s_t)
  is shared by the forward tap (pixel g gathering from g+t) and the backward
  tap (pixel g+t gathering from g).  With m_t = W_t*(xp[g+t]-xp[g]):
      S0 = s_c + sum_t (W_t[g] + W_t[g-t]),  S1 = sum_t (m_t[g] - m_t[g-t])
      out = x + S1/S0        (S0 >= s_c > 0: the 1e-10 clip never binds)
  Reflect padding makes the shared-weight trick exact at image borders.

Implementation notes:
  * Channels flattened along rows: xp [Rpad, 516] fp16 (host-padded).
    13 strips of 128 consecutive padded rows, processed in GROUPS OF 4:
    each field op (d/sq/exp/m) covers all 4 strips via a 2-level AP
    [[slot, 4], [1, wd]], amortizing the per-instruction fixed overhead
    (58cyc DVE / 224cyc ACT bubble) while keeping per-offset granularity
    so the engines pipeline offset-by-offset.
  * 3 row-shifted slab copies T0..T2 are DMA-loaded per group so every
    compute operand starts at partition 0 (HW rule: start partition must
    be 0/32/64/96); row shifts on computed fields instead run on the
    otherwise-idle TensorE as matmuls with host-provided +-shifted
    identity matrices, accumulating S0/S1 per strip in PSUM fp32
    (8 accumulators = all 8 PSUM banks).  dj==0 offsets use combined
    (M2 +- M_{2-di}) matrices: forward+backward in one matmul.
  * Fields are fp16 (DVE tensor ops at 2x); Square runs 6 offsets on
    ScalarE / 6 on VectorE (engine balance); Exp folds the -50 scale and
    ln(spatial) bias into ACT's free affine; s_center enters S0 via
    s_c*I @ ones.  Epilogue: custom-DVE fast reciprocal; result fp32.
"""

import numpy as np

B, C, H, W = 8, 3, 512, 512
SIGMA_R = 0.1
EXP_SCALE = -1.0 / (2.0 * SIGMA_R * SIGMA_R)  # -50

HALF_OFFSETS = [
    (0, 1), (0, 2),
    (1, -2), (1, -1), (1, 0), (1, 1), (1, 2),
    (2, -2), (2, -1), (2, 0), (2, 1), (2, 2),
]
SQ_ON_ACT = {(0, 2), (1, -2), (1, 0), (1, 2), (2, -1), (2, 1)}
GSZ = 4

_CACHE = {}


def _strip_plan():
    Hp = H + 4
    R = C * Hp
    strips = []
    rbase = 0
    while R - 4 - rbase > 0:
        strips.append((rbase, min(124, R - 4 - rbase)))
        rbase += 124
    return strips[-1][0] + 132, strips


def _build():
    from contextlib import ExitStack

    import concourse.bacc as bacc
    import concourse.bass as bass
    import concourse.tile as tile
    from concourse import mybir

    F32 = mybir.dt.float32
    F16 = mybir.dt.float16
    Alu = mybir.AluOpType
    Act = mybir.ActivationFunctionType

    Hp, Wp = H + 4, W + 4
    R = C * Hp
    Rpad, strips = _strip_plan()
    NS = Wp  # per-strip slot width inside group tiles

    def pairap(v, col0, nh, wd):
        return bass.AP(tensor=v.tensor, offset=v.offset + col0,
                       ap=[v.ap[0], [NS, nh], [1, wd]])

    nc = bacc.Bacc(
        "TRN2",
        target_bir_lowering=False,
        debug=False,
        enable_asserts=False,
        num_devices=B,
    )
    xp = nc.dram_tensor("xp", [Rpad, Wp], F16, kind="ExternalInput").ap()
    lsp = nc.dram_tensor("lsp", [1, 25], F32, kind="ExternalInput").ap()
    shm = nc.dram_tensor("shm", [11 * 128, 128], F16, kind="ExternalInput").ap()
    y = nc.dram_tensor("y", [C * H, W], F32, kind="ExternalOutput").ap()

    with tile.TileContext(nc) as tc, ExitStack() as ctx:
        consts = ctx.enter_context(tc.tile_pool(name="consts", bufs=1))
        lt = consts.tile([128, 25], F32)
        nc.gpsimd.dma_start(out=lt[:], in_=lsp.to_broadcast([128, 25]))
        M = []  # [M0,M1,M2, N0,N1,N2, SC, C1,C0, D1,D0]
        for s in range(11):
            m_ = consts.tile([128, 128], F16, name=f"shm{s}", tag=f"shm{s}")
            nc.sync.dma_start(out=m_[:, :], in_=shm[s * 128:(s + 1) * 128, :])
            M.append(m_)
        CMB = {1: (M[7], M[9]), 2: (M[8], M[10])}
        ones = consts.tile([128, W], F16)
        nc.vector.memset(ones[:], 1.0)

        slabs = ctx.enter_context(tc.tile_pool(name="slabs", bufs=2))
        fld = ctx.enter_context(tc.tile_pool(name="fld", bufs=8))
        accp = ctx.enter_context(tc.tile_pool(name="accum", bufs=2))
        psum = ctx.enter_context(tc.tile_pool(name="psum", bufs=1, space="PSUM"))

        groups = [strips[i:i + GSZ] for i in range(0, len(strips), GSZ)]
        for grp in groups:
            nh = len(grp)
            T = [slabs.tile([128, nh * NS], F16, tag=f"T{v}", name=f"T{v}")
                 for v in range(3)]
            for v in range(3):
                for h, (rbase, K) in enumerate(grp):
                    nc.sync.dma_start(
                        out=T[v][:, h * NS:h * NS + Wp],
                        in_=xp[rbase + v:rbase + v + 128, :])

            S0 = [psum.tile([128, W], F32, tag=f"S0p{h}", name=f"S0p{h}")
                  for h in range(nh)]
            S1 = [psum.tile([128, W], F32, tag=f"S1p{h}", name=f"S1p{h}")
                  for h in range(nh)]
            for h in range(nh):
                nc.tensor.matmul(S0[h][:, :], M[6][:, :], ones[:, :],
                                 start=True, stop=False)

            for oi, (di, dj) in enumerate(HALF_OFFSETS):
                cl = min(0, -dj)
                ch = max(W, W - dj)
                wd = ch - cl
                t_bias = (di + 2) * 5 + (dj + 2)

                dt = fld.tile([128, nh * NS], F16, tag="dt", name="dt")
                nc.vector.tensor_tensor(
                    out=pairap(dt[:, :], cl + 2, nh, wd),
                    in0=pairap(T[di][:, :], cl + dj + 2, nh, wd),
                    in1=pairap(T[0][:, :], cl + 2, nh, wd),
                    op=Alu.subtract)
                sq = fld.tile([128, nh * NS], F16, tag="sq", name="sq")
                if (di, dj) in SQ_ON_ACT:
                    nc.scalar.activation(
                        pairap(sq[:, :], cl + 2, nh, wd),
                        pairap(dt[:, :], cl + 2, nh, wd), Act.Square)
                else:
                    nc.vector.tensor_tensor(
                        out=pairap(sq[:, :], cl + 2, nh, wd),
                        in0=pairap(dt[:, :], cl + 2, nh, wd),
                        in1=pairap(dt[:, :], cl + 2, nh, wd),
                        op=Alu.mult)
                Wt = fld.tile([128, nh * NS], F16, tag="Wt", name="Wt")
                nc.scalar.activation(
                    pairap(Wt[:, :], cl + 2, nh, wd),
                    pairap(sq[:, :], cl + 2, nh, wd),
                    Act.Exp, bias=lt[:, t_bias:t_bias + 1], scale=EXP_SCALE)
                mt = fld.tile([128, nh * NS], F16, tag="mt", name="mt")
                nc.vector.tensor_tensor(
                    out=pairap(mt[:, :], cl + 2, nh, wd),
                    in0=pairap(Wt[:, :], cl + 2, nh, wd),
                    in1=pairap(dt[:, :], cl + 2, nh, wd),
                    op=Alu.mult)

                fw = 2          # tile col of output j=0, forward (j'+2)
                bw = 2 - dj     # backward
                last = oi == len(HALF_OFFSETS) - 1
                for h in range(nh):
                    o = h * NS
                    if dj == 0 and di in CMB:
                        c0, d0 = CMB[di]
                        nc.tensor.matmul(S0[h][:, :], c0[:, :],
                                         Wt[:, o + fw:o + fw + W],
                                         start=False, stop=last)
                        nc.tensor.matmul(S1[h][:, :], d0[:, :],
                                         mt[:, o + fw:o + fw + W],
                                         start=(oi == 0), stop=last)
                    else:
                        nc.tensor.matmul(S0[h][:, :], M[2][:, :],
                                         Wt[:, o + fw:o + fw + W],
                                         start=False, stop=False)
                        nc.tensor.matmul(S0[h][:, :], M[2 - di][:, :],
                                         Wt[:, o + bw:o + bw + W],
                                         start=False, stop=last)
                        nc.tensor.matmul(S1[h][:, :], M[2][:, :],
                                         mt[:, o + fw:o + fw + W],
                                         start=(oi == 0), stop=False)
                        nc.tensor.matmul(S1[h][:, :], M[3 + 2 - di][:, :],
                                         mt[:, o + bw:o + bw + W],
                                         start=False, stop=last)

            for h, (rbase, K) in enumerate(grp):
                Rc = accp.tile([128, W], F32, tag="Rc", name="Rc")
                nc.vector.reciprocal_approx_fast(out=Rc[:K, :],
                                                 in_=S0[h][:K, :])
                tmp = accp.tile([128, W], F32, tag="tmp", name="tmp")
                nc.vector.tensor_tensor(
                    out=tmp[:K, :], in0=S1[h][:K, :], in1=Rc[:K, :],
                    op=Alu.mult)
                res = accp.tile([128, W], F32, tag="res", name="res")
                nc.vector.tensor_tensor(
                    out=res[:K, :], in0=tmp[:K, :],
                    in1=T[2][0:K, h * NS + 2:h * NS + 2 + W], op=Alu.add)

                k = 0
                while k < K:
                    g = rbase + 2 + k
                    if g < R and 2 <= (g % Hp) <= Hp - 3:
                        k1 = k
                        while k1 < K:
                            g1 = rbase + 2 + k1
                            if g1 >= R or not (2 <= (g1 % Hp) <= Hp - 3):
                                break
                            if (g1 % Hp) == 2 and k1 > k:
                                break
                            k1 += 1
                        h0 = (g // Hp) * H + (g % Hp) - 2
                        nc.sync.dma_start(out=y[h0:h0 + (k1 - k), :],
                                          in_=res[k:k1, :])
                        k = k1
                    else:
                        k += 1

    nc.compile()
    return nc


def _get_module():
    if "nc" not in _CACHE:
        _CACHE["nc"] = _build()
    return _CACHE["nc"]


def _pack_core(xc):
    """xc [C,H,W] f32 -> reflect-padded fp16 [Rpad, W+4]."""
    Rpad, _ = _strip_plan()
    xpad = np.pad(xc, ((0, 0), (2, 2), (2, 2)), mode="reflect")
    flat = xpad.reshape(C * (H + 4), W + 4)
    extra = Rpad - flat.shape[0]
    if extra > 0:
        flat = np.concatenate([flat, np.repeat(flat[-1:], extra, axis=0)],
                              axis=0)
    return np.ascontiguousarray(flat, dtype=np.float16)


def kernel(x, spatial, _trace=False):
    from concourse.bass_utils import run_bass_kernel_spmd

    x = np.asarray(x, dtype=np.float32)
    spatial = np.asarray(spatial, dtype=np.float32)
    assert x.shape == (B, C, H, W) and spatial.shape == (5, 5)
    # weight-field sharing between forward/backward taps needs symmetry
    assert np.allclose(spatial, spatial[::-1, ::-1], rtol=1e-5), \
        "kernel assumes point-symmetric spatial weights"

    lsp = np.log(spatial).reshape(1, 25).astype(np.float32)
    E = [np.eye(128, 128, k=-s, dtype=np.float32) for s in range(3)]
    mats = [e.astype(np.float16) for e in E]
    mats += [(-e).astype(np.float16) for e in E]
    mats.append(np.eye(128, dtype=np.float16)
                * np.float16(float(spatial[2, 2])))
    mats.append((E[2] + E[1]).astype(np.float16))  # C1: di=1 S0 combined
    mats.append((E[2] + E[0]).astype(np.float16))  # C0: di=2 S0 combined
    mats.append((E[2] - E[1]).astype(np.float16))  # D1: di=1 S1 combined
    mats.append((E[2] - E[0]).astype(np.float16))  # D0: di=2 S1 combined
    shm = np.concatenate(mats, axis=0)

    nc = _get_module()
    in_maps = [{"xp": _pack_core(x[b]), "lsp": lsp, "shm": shm}
               for b in range(B)]
    res = run_bass_kernel_spmd(nc, in_maps, core_ids=list(range(B)),
                               trace=_trace)
    out = np.stack([res.results[b]["y"].reshape(C, H, W) for b in range(B)])
    if _trace:
        return out.astype(np.float32), res
    return out.astype(np.float32)


# revision 5
# speedup vs baseline: 1.0735x; 1.0049x over previous
"""Bilateral filter denoiser (5x5, sigma_s=2.0, sigma_r=0.1) on 8 Trainium2
NeuronCores.  Takes full inputs x (8,3,512,512) f32 + spatial (5,5) f32;
pure data parallel: one batch element per core; returns the full output.

Per-core kernel (Bass/Tile), symmetric half-offset formulation:
  For each of the 12 half offsets t=(di,dj) the range/spatial weight field
      W_t[g] = exp(-50*(xp[g+t]-xp[g])^2 + ln s_t)
  is shared by the forward tap (pixel g gathering from g+t) and the backward
  tap (pixel g+t gathering from g).  With m_t = W_t*(xp[g+t]-xp[g]):
      S0 = s_c + sum_t (W_t[g] + W_t[g-t]),  S1 = sum_t (m_t[g] - m_t[g-t])
      out = x + S1/S0        (S0 >= s_c > 0: the 1e-10 clip never binds)
  Reflect padding makes the shared-weight trick exact at image borders.

Implementation notes:
  * Channels flattened along rows: xp [Rpad, 516] fp16 (host-padded).
    13 strips of 128 consecutive padded rows, processed in GROUPS OF 4:
    each field op (d/sq/exp/m) covers all 4 strips via a 2-level AP
    [[slot, 4], [1, wd]], amortizing the per-instruction fixed overhead
    (58cyc DVE / 224cyc ACT bubble) while keeping per-offset granularity
    so the engines pipeline offset-by-offset.
  * 3 row-shifted slab copies T0..T2 are DMA-loaded per group so every
    compute operand starts at partition 0 (HW rule: start partition must
    be 0/32/64/96); row shifts on computed fields instead run on the
    otherwise-idle TensorE as matmuls with host-provided +-shifted
    identity matrices, accumulating S0/S1 per strip in PSUM fp32
    (8 accumulators = all 8 PSUM banks).  dj==0 offsets use combined
    (M2 +- M_{2-di}) matrices: forward+backward in one matmul.
  * Fields are fp16 (DVE tensor ops at 2x); Square runs 6 offsets on
    ScalarE / 6 on VectorE (engine balance); Exp folds the -50 scale and
    ln(spatial) bias into ACT's free affine; s_center enters S0 via
    s_c*I @ ones.  Epilogue: custom-DVE fast reciprocal; result fp32.
"""

import numpy as np

B, C, H, W = 8, 3, 512, 512
SIGMA_R = 0.1
EXP_SCALE = -1.0 / (2.0 * SIGMA_R * SIGMA_R)  # -50

HALF_OFFSETS = [
    (0, 1), (0, 2),
    (1, -2), (1, -1), (1, 0), (1, 1), (1, 2),
    (2, -2), (2, -1), (2, 0), (2, 1), (2, 2),
]
SQ_ON_ACT = {(0, 2), (1, -2), (1, 0), (1, 2), (2, -1), (2, 1)}
GSZ = 4

_CACHE = {}


def _strip_plan():
    Hp = H + 4
    R = C * Hp
    strips = []
    rbase = 0
    while R - 4 - rbase > 0:
        strips.append((rbase, min(124, R - 4 - rbase)))
        rbase += 124
    return strips[-1][0] + 132, strips


def _build():
    from contextlib import ExitStack

    import concourse.bacc as bacc
    import concourse.bass as bass
    import concourse.tile as tile
    from concourse import mybir

    F32 = mybir.dt.float32
    F16 = mybir.dt.float16
    Alu = mybir.AluOpType
    Act = mybir.ActivationFunctionType

    Hp, Wp = H + 4, W + 4
    R = C * Hp
    Rpad, strips = _strip_plan()
    NS = Wp  # per-strip slot width inside group tiles

    def pairap(v, col0, nh, wd):
        return bass.AP(tensor=v.tensor, offset=v.offset + col0,
                       ap=[v.ap[0], [NS, nh], [1, wd]])

    nc = bacc.Bacc(
        "TRN2",
        target_bir_lowering=False,
        debug=False,
        enable_asserts=False,
        num_devices=B,
    )
    xp = nc.dram_tensor("xp", [Rpad, Wp], F16, kind="ExternalInput").ap()
    lsp = nc.dram_tensor("lsp", [1, 25], F32, kind="ExternalInput").ap()
    shm = nc.dram_tensor("shm", [11 * 128, 128], F16, kind="ExternalInput").ap()
    y = nc.dram_tensor("y", [C * H, W], F32, kind="ExternalOutput").ap()

    with tile.TileContext(nc) as tc, ExitStack() as ctx:
        consts = ctx.enter_context(tc.tile_pool(name="consts", bufs=1))
        lt = consts.tile([128, 25], F32)
        nc.gpsimd.dma_start(out=lt[:], in_=lsp.to_broadcast([128, 25]))
        M = []  # [M0,M1,M2, N0,N1,N2, SC, C1,C0, D1,D0]
        for s in range(11):
            m_ = consts.tile([128, 128], F16, name=f"shm{s}", tag=f"shm{s}")
            nc.sync.dma_start(out=m_[:, :], in_=shm[s * 128:(s + 1) * 128, :])
            M.append(m_)
        CMB = {1: (M[7], M[9]), 2: (M[8], M[10])}
        ones = consts.tile([128, W], F16)
        nc.vector.memset(ones[:], 1.0)

        slabs = ctx.enter_context(tc.tile_pool(name="slabs", bufs=2))
        fld = ctx.enter_context(tc.tile_pool(name="fld", bufs=8))
        accp = ctx.enter_context(tc.tile_pool(name="accum", bufs=2))
        psum = ctx.enter_context(tc.tile_pool(name="psum", bufs=1, space="PSUM"))

        groups = [strips[i:i + GSZ] for i in range(0, len(strips), GSZ)]
        for grp in groups:
            nh = len(grp)
            T = [slabs.tile([128, nh * NS], F16, tag=f"T{v}", name=f"T{v}")
                 for v in range(3)]
            for v in range(3):
                for h, (rbase, K) in enumerate(grp):
                    nc.sync.dma_start(
                        out=T[v][:, h * NS:h * NS + Wp],
                        in_=xp[rbase + v:rbase + v + 128, :])

            S0 = [psum.tile([128, W], F32, tag=f"S0p{h}", name=f"S0p{h}")
                  for h in range(nh)]
            S1 = [psum.tile([128, W], F32, tag=f"S1p{h}", name=f"S1p{h}")
                  for h in range(nh)]
            for h in range(nh):
                nc.tensor.matmul(S0[h][:, :], M[6][:, :], ones[:, :],
                                 start=True, stop=False)

            for oi, (di, dj) in enumerate(HALF_OFFSETS):
                cl = min(0, -dj)
                ch = max(W, W - dj)
                wd = ch - cl
                t_bias = (di + 2) * 5 + (dj + 2)

                dt = fld.tile([128, nh * NS], F16, tag="dt", name="dt")
                nc.vector.tensor_tensor(
                    out=pairap(dt[:, :], cl + 2, nh, wd),
                    in0=pairap(T[di][:, :], cl + dj + 2, nh, wd),
                    in1=pairap(T[0][:, :], cl + 2, nh, wd),
                    op=Alu.subtract)
                sq = fld.tile([128, nh * NS], F16, tag="sq", name="sq")
                if (di, dj) in SQ_ON_ACT:
                    nc.scalar.activation(
                        pairap(sq[:, :], cl + 2, nh, wd),
                        pairap(dt[:, :], cl + 2, nh, wd), Act.Square)
                else:
                    nc.vector.tensor_tensor(
                        out=pairap(sq[:, :], cl + 2, nh, wd),
                        in0=pairap(dt[:, :], cl + 2, nh, wd),
                        in1=pairap(dt[:, :], cl + 2, nh, wd),
                        op=Alu.mult)
                Wt = fld.tile([128, nh * NS], F16, tag="Wt", name="Wt")
                nc.scalar.activation(
                    pairap(Wt[:, :], cl + 2, nh, wd),
                    pairap(sq[:, :], cl + 2, nh, wd),
                    Act.Exp, bias=lt[:, t_bias:t_bias + 1], scale=EXP_SCALE)
                mt = fld.tile([128, nh * NS], F16, tag="mt", name="mt")
                nc.vector.tensor_tensor(
                    out=pairap(mt[:, :], cl + 2, nh, wd),
                    in0=pairap(Wt[:, :], cl + 2, nh, wd),
                    in1=pairap(dt[:, :], cl + 2, nh, wd),
                    op=Alu.mult)

                fw = 2          # tile col of output j=0, forward (j'+2)
                bw = 2 - dj     # backward
                last = oi == len(HALF_OFFSETS) - 1
                for h in range(nh):
                    o = h * NS
                    if dj == 0 and di in CMB:
                        c0, d0 = CMB[di]
                        nc.tensor.matmul(S0[h][:, :], c0[:, :],
                                         Wt[:, o + fw:o + fw + W],
                                         start=False, stop=last)
                        nc.tensor.matmul(S1[h][:, :], d0[:, :],
                                         mt[:, o + fw:o + fw + W],
                                         start=(oi == 0), stop=last)
                    else:
                        nc.tensor.matmul(S0[h][:, :], M[2][:, :],
                                         Wt[:, o + fw:o + fw + W],
                                         start=False, stop=False)
                        nc.tensor.matmul(S0[h][:, :], M[2 - di][:, :],
                                         Wt[:, o + bw:o + bw + W],
                                         start=False, stop=last)
                        nc.tensor.matmul(S1[h][:, :], M[2][:, :],
                                         mt[:, o + fw:o + fw + W],
                                         start=(oi == 0), stop=False)
                        nc.tensor.matmul(S1[h][:, :], M[3 + 2 - di][:, :],
                                         mt[:, o + bw:o + bw + W],
                                         start=False, stop=last)

            for h, (rbase, K) in enumerate(grp):
                # evacuate PSUM via ScalarE (slack engine) so the banks free
                # for the next group's matmuls before the VectorE epilogue
                S0s = accp.tile([128, W], F32, tag="S0s", name="S0s")
                nc.scalar.copy(S0s[:K, :], S0[h][:K, :])
                S1s = accp.tile([128, W], F32, tag="S1s", name="S1s")
                nc.scalar.copy(S1s[:K, :], S1[h][:K, :])
                Rc = accp.tile([128, W], F32, tag="Rc", name="Rc")
                nc.vector.reciprocal_approx_fast(out=Rc[:K, :],
                                                 in_=S0s[:K, :])
                tmp = accp.tile([128, W], F32, tag="tmp", name="tmp")
                nc.vector.tensor_tensor(
                    out=tmp[:K, :], in0=S1s[:K, :], in1=Rc[:K, :],
                    op=Alu.mult)
                res = accp.tile([128, W], F32, tag="res", name="res")
                nc.vector.tensor_tensor(
                    out=res[:K, :], in0=tmp[:K, :],
                    in1=T[2][0:K, h * NS + 2:h * NS + 2 + W], op=Alu.add)

                k = 0
                while k < K:
                    g = rbase + 2 + k
                    if g < R and 2 <= (g % Hp) <= Hp - 3:
                        k1 = k
                        while k1 < K:
                            g1 = rbase + 2 + k1
                            if g1 >= R or not (2 <= (g1 % Hp) <= Hp - 3):
                                break
                            if (g1 % Hp) == 2 and k1 > k:
                                break
                            k1 += 1
                        h0 = (g // Hp) * H + (g % Hp) - 2
                        nc.sync.dma_start(out=y[h0:h0 + (k1 - k), :],
                                          in_=res[k:k1, :])
                        k = k1
                    else:
                        k += 1

    nc.compile()
    return nc


def _get_module():
    if "nc" not in _CACHE:
        _CACHE["nc"] = _build()
    return _CACHE["nc"]


def _pack_core(xc):
    """xc [C,H,W] f32 -> reflect-padded fp16 [Rpad, W+4]."""
    Rpad, _ = _strip_plan()
    xpad = np.pad(xc, ((0, 0), (2, 2), (2, 2)), mode="reflect")
    flat = xpad.reshape(C * (H + 4), W + 4)
    extra = Rpad - flat.shape[0]
    if extra > 0:
        flat = np.concatenate([flat, np.repeat(flat[-1:], extra, axis=0)],
                              axis=0)
    return np.ascontiguousarray(flat, dtype=np.float16)


def kernel(x, spatial, _trace=False):
    from concourse.bass_utils import run_bass_kernel_spmd

    x = np.asarray(x, dtype=np.float32)
    spatial = np.asarray(spatial, dtype=np.float32)
    assert x.shape == (B, C, H, W) and spatial.shape == (5, 5)
    # weight-field sharing between forward/backward taps needs symmetry
    assert np.allclose(spatial, spatial[::-1, ::-1], rtol=1e-5), \
        "kernel assumes point-symmetric spatial weights"

    lsp = np.log(spatial).reshape(1, 25).astype(np.float32)
    E = [np.eye(128, 128, k=-s, dtype=np.float32) for s in range(3)]
    mats = [e.astype(np.float16) for e in E]
    mats += [(-e).astype(np.float16) for e in E]
    mats.append(np.eye(128, dtype=np.float16)
                * np.float16(float(spatial[2, 2])))
    mats.append((E[2] + E[1]).astype(np.float16))  # C1: di=1 S0 combined
    mats.append((E[2] + E[0]).astype(np.float16))  # C0: di=2 S0 combined
    mats.append((E[2] - E[1]).astype(np.float16))  # D1: di=1 S1 combined
    mats.append((E[2] - E[0]).astype(np.float16))  # D0: di=2 S1 combined
    shm = np.concatenate(mats, axis=0)

    nc = _get_module()
    in_maps = [{"xp": _pack_core(x[b]), "lsp": lsp, "shm": shm}
               for b in range(B)]
    res = run_bass_kernel_spmd(nc, in_maps, core_ids=list(range(B)),
                               trace=_trace)
    out = np.stack([res.results[b]["y"].reshape(C, H, W) for b in range(B)])
    if _trace:
        return out.astype(np.float32), res
    return out.astype(np.float32)


# revision 6
# speedup vs baseline: 1.0760x; 1.0024x over previous
"""Bilateral filter denoiser (5x5, sigma_s=2.0, sigma_r=0.1) on 8 Trainium2
NeuronCores.  Takes full inputs x (8,3,512,512) f32 + spatial (5,5) f32;
pure data parallel: one batch element per core; returns the full output.

Per-core kernel (Bass/Tile), symmetric half-offset formulation:
  For each of the 12 half offsets t=(di,dj) the range/spatial weight field
      W_t[g] = exp(-50*(xp[g+t]-xp[g])^2 + ln s_t)
  is shared by the forward tap (pixel g gathering from g+t) and the backward
  tap (pixel g+t gathering from g).  With m_t = W_t*(xp[g+t]-xp[g]):
      S0 = s_c + sum_t (W_t[g] + W_t[g-t]),  S1 = sum_t (m_t[g] - m_t[g-t])
      out = x + S1/S0        (S0 >= s_c > 0: the 1e-10 clip never binds)
  Reflect padding makes the shared-weight trick exact at image borders.

Implementation notes:
  * Channels flattened along rows: xp [Rpad, 516] fp16 (host-padded).
    13 strips of 128 consecutive padded rows, processed in GROUPS OF 4:
    each field op (d/sq/exp/m) covers all 4 strips via a 2-level AP
    [[slot, 4], [1, wd]], amortizing the per-instruction fixed overhead
    (58cyc DVE / 224cyc ACT bubble) while keeping per-offset granularity
    so the engines pipeline offset-by-offset.
  * 3 row-shifted slab copies T0..T2 are DMA-loaded per group so every
    compute operand starts at partition 0 (HW rule: start partition must
    be 0/32/64/96); row shifts on computed fields instead run on the
    otherwise-idle TensorE as matmuls with host-provided +-shifted
    identity matrices, accumulating S0/S1 per strip in PSUM fp32
    (8 accumulators = all 8 PSUM banks).  dj==0 offsets use combined
    (M2 +- M_{2-di}) matrices: forward+backward in one matmul.
  * Fields are fp16 (DVE tensor ops at 2x); Square runs 6 offsets on
    ScalarE / 6 on VectorE (engine balance); Exp folds the -50 scale and
    ln(spatial) bias into ACT's free affine; s_center enters S0 via
    s_c*I @ ones.  Epilogue: custom-DVE fast reciprocal; result fp32.
"""

import numpy as np

B, C, H, W = 8, 3, 512, 512
SIGMA_R = 0.1
EXP_SCALE = -1.0 / (2.0 * SIGMA_R * SIGMA_R)  # -50

HALF_OFFSETS = [
    (0, 1), (0, 2),
    (1, -2), (1, -1), (1, 0), (1, 1), (1, 2),
    (2, -2), (2, -1), (2, 0), (2, 1), (2, 2),
]
SQ_ON_ACT = {(0, 2), (1, -2), (1, 0), (1, 2), (2, -1), (2, 1)}
GSZ = 4

_CACHE = {}


def _strip_plan():
    Hp = H + 4
    R = C * Hp
    strips = []
    rbase = 0
    while R - 4 - rbase > 0:
        strips.append((rbase, min(124, R - 4 - rbase)))
        rbase += 124
    return strips[-1][0] + 132, strips


def _build():
    from contextlib import ExitStack

    import concourse.bacc as bacc
    import concourse.bass as bass
    import concourse.tile as tile
    from concourse import mybir

    F32 = mybir.dt.float32
    F16 = mybir.dt.float16
    Alu = mybir.AluOpType
    Act = mybir.ActivationFunctionType

    Hp, Wp = H + 4, W + 4
    R = C * Hp
    Rpad, strips = _strip_plan()
    NS = Wp  # per-strip slot width inside group tiles

    def pairap(v, col0, nh, wd):
        return bass.AP(tensor=v.tensor, offset=v.offset + col0,
                       ap=[v.ap[0], [NS, nh], [1, wd]])

    nc = bacc.Bacc(
        "TRN2",
        target_bir_lowering=False,
        debug=False,
        enable_asserts=False,
        num_devices=B,
    )
    xp = nc.dram_tensor("xp", [Rpad, Wp], F16, kind="ExternalInput").ap()
    lsp = nc.dram_tensor("lsp", [1, 25], F32, kind="ExternalInput").ap()
    shm = nc.dram_tensor("shm", [11 * 128, 128], F16, kind="ExternalInput").ap()
    y = nc.dram_tensor("y", [C * H, W], F32, kind="ExternalOutput").ap()

    with tile.TileContext(nc) as tc, ExitStack() as ctx:
        consts = ctx.enter_context(tc.tile_pool(name="consts", bufs=1))
        lt = consts.tile([128, 25], F32)
        nc.gpsimd.dma_start(out=lt[:], in_=lsp.to_broadcast([128, 25]))
        M = []  # [M0,M1,M2, N0,N1,N2, SC, C1,C0, D1,D0]
        for s in range(11):
            m_ = consts.tile([128, 128], F16, name=f"shm{s}", tag=f"shm{s}")
            nc.sync.dma_start(out=m_[:, :], in_=shm[s * 128:(s + 1) * 128, :])
            M.append(m_)
        CMB = {1: (M[7], M[9]), 2: (M[8], M[10])}
        ones = consts.tile([128, W], F16)
        nc.vector.memset(ones[:], 1.0)

        slabs = ctx.enter_context(tc.tile_pool(name="slabs", bufs=2))
        fld = ctx.enter_context(tc.tile_pool(name="fld", bufs=8))
        accp = ctx.enter_context(tc.tile_pool(name="accum", bufs=3))
        psum = ctx.enter_context(tc.tile_pool(name="psum", bufs=1, space="PSUM"))

        groups = [strips[i:i + GSZ] for i in range(0, len(strips), GSZ)]
        for grp in groups:
            nh = len(grp)
            T = [slabs.tile([128, nh * NS], F16, tag=f"T{v}", name=f"T{v}")
                 for v in range(3)]
            for v in range(3):
                for h, (rbase, K) in enumerate(grp):
                    nc.sync.dma_start(
                        out=T[v][:, h * NS:h * NS + Wp],
                        in_=xp[rbase + v:rbase + v + 128, :])

            S0 = [psum.tile([128, W], F32, tag=f"S0p{h}", name=f"S0p{h}")
                  for h in range(nh)]
            S1 = [psum.tile([128, W], F32, tag=f"S1p{h}", name=f"S1p{h}")
                  for h in range(nh)]
            for h in range(nh):
                nc.tensor.matmul(S0[h][:, :], M[6][:, :], ones[:, :],
                                 start=True, stop=False)

            for oi, (di, dj) in enumerate(HALF_OFFSETS):
                cl = min(0, -dj)
                ch = max(W, W - dj)
                wd = ch - cl
                t_bias = (di + 2) * 5 + (dj + 2)

                dt = fld.tile([128, nh * NS], F16, tag="dt", name="dt")
                nc.vector.tensor_tensor(
                    out=pairap(dt[:, :], cl + 2, nh, wd),
                    in0=pairap(T[di][:, :], cl + dj + 2, nh, wd),
                    in1=pairap(T[0][:, :], cl + 2, nh, wd),
                    op=Alu.subtract)
                sq = fld.tile([128, nh * NS], F16, tag="sq", name="sq")
                if (di, dj) in SQ_ON_ACT:
                    nc.scalar.activation(
                        pairap(sq[:, :], cl + 2, nh, wd),
                        pairap(dt[:, :], cl + 2, nh, wd), Act.Square)
                else:
                    nc.vector.tensor_tensor(
                        out=pairap(sq[:, :], cl + 2, nh, wd),
                        in0=pairap(dt[:, :], cl + 2, nh, wd),
                        in1=pairap(dt[:, :], cl + 2, nh, wd),
                        op=Alu.mult)
                Wt = fld.tile([128, nh * NS], F16, tag="Wt", name="Wt")
                nc.scalar.activation(
                    pairap(Wt[:, :], cl + 2, nh, wd),
                    pairap(sq[:, :], cl + 2, nh, wd),
                    Act.Exp, bias=lt[:, t_bias:t_bias + 1], scale=EXP_SCALE)
                mt = fld.tile([128, nh * NS], F16, tag="mt", name="mt")
                nc.vector.tensor_tensor(
                    out=pairap(mt[:, :], cl + 2, nh, wd),
                    in0=pairap(Wt[:, :], cl + 2, nh, wd),
                    in1=pairap(dt[:, :], cl + 2, nh, wd),
                    op=Alu.mult)

                fw = 2          # tile col of output j=0, forward (j'+2)
                bw = 2 - dj     # backward
                last = oi == len(HALF_OFFSETS) - 1
                for h in range(nh):
                    o = h * NS
                    if dj == 0 and di in CMB:
                        c0, d0 = CMB[di]
                        nc.tensor.matmul(S0[h][:, :], c0[:, :],
                                         Wt[:, o + fw:o + fw + W],
                                         start=False, stop=last)
                        nc.tensor.matmul(S1[h][:, :], d0[:, :],
                                         mt[:, o + fw:o + fw + W],
                                         start=(oi == 0), stop=last)
                    else:
                        nc.tensor.matmul(S0[h][:, :], M[2][:, :],
                                         Wt[:, o + fw:o + fw + W],
                                         start=False, stop=False)
                        nc.tensor.matmul(S0[h][:, :], M[2 - di][:, :],
                                         Wt[:, o + bw:o + bw + W],
                                         start=False, stop=last)
                        nc.tensor.matmul(S1[h][:, :], M[2][:, :],
                                         mt[:, o + fw:o + fw + W],
                                         start=(oi == 0), stop=False)
                        nc.tensor.matmul(S1[h][:, :], M[3 + 2 - di][:, :],
                                         mt[:, o + bw:o + bw + W],
                                         start=False, stop=last)

            for h, (rbase, K) in enumerate(grp):
                # evacuate PSUM via ScalarE (slack engine) so the banks free
                # for the next group's matmuls before the VectorE epilogue
                S0s = accp.tile([128, W], F32, tag="S0s", name="S0s")
                nc.scalar.copy(S0s[:K, :], S0[h][:K, :])
                S1s = accp.tile([128, W], F32, tag="S1s", name="S1s")
                nc.scalar.copy(S1s[:K, :], S1[h][:K, :])
                Rc = accp.tile([128, W], F32, tag="Rc", name="Rc")
                nc.vector.reciprocal_approx_fast(out=Rc[:K, :],
                                                 in_=S0s[:K, :])
                tmp = accp.tile([128, W], F32, tag="tmp", name="tmp")
                nc.vector.tensor_tensor(
                    out=tmp[:K, :], in0=S1s[:K, :], in1=Rc[:K, :],
                    op=Alu.mult)
                res = accp.tile([128, W], F32, tag="res", name="res")
                nc.vector.tensor_tensor(
                    out=res[:K, :], in0=tmp[:K, :],
                    in1=T[2][0:K, h * NS + 2:h * NS + 2 + W], op=Alu.add)

                k = 0
                while k < K:
                    g = rbase + 2 + k
                    if g < R and 2 <= (g % Hp) <= Hp - 3:
                        k1 = k
                        while k1 < K:
                            g1 = rbase + 2 + k1
                            if g1 >= R or not (2 <= (g1 % Hp) <= Hp - 3):
                                break
                            if (g1 % Hp) == 2 and k1 > k:
                                break
                            k1 += 1
                        h0 = (g // Hp) * H + (g % Hp) - 2
                        nc.sync.dma_start(out=y[h0:h0 + (k1 - k), :],
                                          in_=res[k:k1, :])
                        k = k1
                    else:
                        k += 1

    nc.compile()
    return nc


def _get_module():
    if "nc" not in _CACHE:
        _CACHE["nc"] = _build()
    return _CACHE["nc"]


def _pack_core(xc):
    """xc [C,H,W] f32 -> reflect-padded fp16 [Rpad, W+4]."""
    Rpad, _ = _strip_plan()
    xpad = np.pad(xc, ((0, 0), (2, 2), (2, 2)), mode="reflect")
    flat = xpad.reshape(C * (H + 4), W + 4)
    extra = Rpad - flat.shape[0]
    if extra > 0:
        flat = np.concatenate([flat, np.repeat(flat[-1:], extra, axis=0)],
                              axis=0)
    return np.ascontiguousarray(flat, dtype=np.float16)


def kernel(x, spatial, _trace=False):
    from concourse.bass_utils import run_bass_kernel_spmd

    x = np.asarray(x, dtype=np.float32)
    spatial = np.asarray(spatial, dtype=np.float32)
    assert x.shape == (B, C, H, W) and spatial.shape == (5, 5)
    # weight-field sharing between forward/backward taps needs symmetry
    assert np.allclose(spatial, spatial[::-1, ::-1], rtol=1e-5), \
        "kernel assumes point-symmetric spatial weights"

    lsp = np.log(spatial).reshape(1, 25).astype(np.float32)
    E = [np.eye(128, 128, k=-s, dtype=np.float32) for s in range(3)]
    mats = [e.astype(np.float16) for e in E]
    mats += [(-e).astype(np.float16) for e in E]
    mats.append(np.eye(128, dtype=np.float16)
                * np.float16(float(spatial[2, 2])))
    mats.append((E[2] + E[1]).astype(np.float16))  # C1: di=1 S0 combined
    mats.append((E[2] + E[0]).astype(np.float16))  # C0: di=2 S0 combined
    mats.append((E[2] - E[1]).astype(np.float16))  # D1: di=1 S1 combined
    mats.append((E[2] - E[0]).astype(np.float16))  # D0: di=2 S1 combined
    shm = np.concatenate(mats, axis=0)

    nc = _get_module()
    in_maps = [{"xp": _pack_core(x[b]), "lsp": lsp, "shm": shm}
               for b in range(B)]
    res = run_bass_kernel_spmd(nc, in_maps, core_ids=list(range(B)),
                               trace=_trace)
    out = np.stack([res.results[b]["y"].reshape(C, H, W) for b in range(B)])
    if _trace:
        return out.astype(np.float32), res
    return out.astype(np.float32)


# revision 7
# speedup vs baseline: 1.0933x; 1.0161x over previous
"""Bilateral filter denoiser (5x5, sigma_s=2.0, sigma_r=0.1) on 8 Trainium2
NeuronCores.  Takes full inputs x (8,3,512,512) f32 + spatial (5,5) f32;
pure data parallel: one batch element per core; returns the full output.

Per-core kernel (Bass/Tile), symmetric half-offset formulation:
  For each of the 12 half offsets t=(di,dj) the range/spatial weight field
      W_t[g] = exp(-50*(xp[g+t]-xp[g])^2 + ln s_t)
  is shared by the forward tap (pixel g gathering from g+t) and the backward
  tap (pixel g+t gathering from g).  With m_t = W_t*(xp[g+t]-xp[g]):
      S0 = s_c + sum_t (W_t[g] + W_t[g-t]),  S1 = sum_t (m_t[g] - m_t[g-t])
      out = x + S1/S0        (S0 >= s_c > 0: the 1e-10 clip never binds)
  Reflect padding makes the shared-weight trick exact at image borders.

Implementation notes:
  * Channels flattened along rows: xp [Rpad, 516] fp16 (host-padded).
    13 strips of 128 consecutive padded rows, processed in GROUPS OF 4:
    each field op (d/sq/exp/m) covers all 4 strips via a 2-level AP
    [[slot, 4], [1, wd]], amortizing the per-instruction fixed overhead
    (58cyc DVE / 224cyc ACT bubble) while keeping per-offset granularity
    so the engines pipeline offset-by-offset.
  * 3 row-shifted slab copies T0..T2 are DMA-loaded per group so every
    compute operand starts at partition 0 (HW rule: start partition must
    be 0/32/64/96); row shifts on computed fields instead run on the
    otherwise-idle TensorE as matmuls with host-provided +-shifted
    identity matrices, accumulating S0/S1 per strip in PSUM fp32
    (8 accumulators = all 8 PSUM banks).  dj==0 offsets use combined
    (M2 +- M_{2-di}) matrices: forward+backward in one matmul.
  * Fields are fp16 (DVE tensor ops at 2x); Square runs 6 offsets on
    ScalarE / 6 on VectorE (engine balance); Exp folds the -50 scale and
    ln(spatial) bias into ACT's free affine; s_center enters S0 via
    s_c*I @ ones.  Epilogue: custom-DVE fast reciprocal; result fp32.
"""

import numpy as np

B, C, H, W = 8, 3, 512, 512
SIGMA_R = 0.1
EXP_SCALE = -1.0 / (2.0 * SIGMA_R * SIGMA_R)  # -50

HALF_OFFSETS = [
    (0, 1), (0, 2),
    (1, -2), (1, -1), (1, 0), (1, 1), (1, 2),
    (2, -2), (2, -1), (2, 0), (2, 1), (2, 2),
]
SQ_ON_ACT = {(0, 2), (1, -2), (1, 0), (1, 2), (2, -1)}
GSZ = 4

_CACHE = {}


def _strip_plan():
    Hp = H + 4
    R = C * Hp
    strips = []
    rbase = 0
    while R - 4 - rbase > 0:
        strips.append((rbase, min(124, R - 4 - rbase)))
        rbase += 124
    return strips[-1][0] + 132, strips


def _build():
    from contextlib import ExitStack

    import concourse.bacc as bacc
    import concourse.bass as bass
    import concourse.tile as tile
    from concourse import mybir

    F32 = mybir.dt.float32
    F16 = mybir.dt.float16
    Alu = mybir.AluOpType
    Act = mybir.ActivationFunctionType

    Hp, Wp = H + 4, W + 4
    R = C * Hp
    Rpad, strips = _strip_plan()
    NS = Wp  # per-strip slot width inside group tiles

    def pairap(v, col0, nh, wd):
        return bass.AP(tensor=v.tensor, offset=v.offset + col0,
                       ap=[v.ap[0], [NS, nh], [1, wd]])

    nc = bacc.Bacc(
        "TRN2",
        target_bir_lowering=False,
        debug=False,
        enable_asserts=False,
        num_devices=B,
    )
    xp = nc.dram_tensor("xp", [Rpad, Wp], F16, kind="ExternalInput").ap()
    lsp = nc.dram_tensor("lsp", [1, 25], F32, kind="ExternalInput").ap()
    shm = nc.dram_tensor("shm", [11 * 128, 128], F16, kind="ExternalInput").ap()
    y = nc.dram_tensor("y", [C * H, W], F32, kind="ExternalOutput").ap()

    with tile.TileContext(nc) as tc, ExitStack() as ctx:
        consts = ctx.enter_context(tc.tile_pool(name="consts", bufs=1))
        lt = consts.tile([128, 25], F32)
        nc.gpsimd.dma_start(out=lt[:], in_=lsp.to_broadcast([128, 25]))
        M = []  # [M0,M1,M2, N0,N1,N2, SC, C1,C0, D1,D0]
        for s in range(11):
            m_ = consts.tile([128, 128], F16, name=f"shm{s}", tag=f"shm{s}")
            nc.sync.dma_start(out=m_[:, :], in_=shm[s * 128:(s + 1) * 128, :])
            M.append(m_)
        CMB = {1: (M[7], M[9]), 2: (M[8], M[10])}
        ones = consts.tile([128, W], F16)
        nc.vector.memset(ones[:], 1.0)
        sc = consts.tile([128, 1], F32)
        nc.scalar.activation(sc[:], lt[:, 12:13], Act.Exp)

        slabs = ctx.enter_context(tc.tile_pool(name="slabs", bufs=2))
        fld = ctx.enter_context(tc.tile_pool(name="fld", bufs=8))
        accp = ctx.enter_context(tc.tile_pool(name="accum", bufs=3))
        psum = ctx.enter_context(tc.tile_pool(name="psum", bufs=1, space="PSUM"))

        groups = [strips[i:i + GSZ] for i in range(0, len(strips), GSZ)]
        for grp in groups:
            nh = len(grp)
            T = [slabs.tile([128, nh * NS], F16, tag=f"T{v}", name=f"T{v}")
                 for v in range(3)]
            for v in range(3):
                for h, (rbase, K) in enumerate(grp):
                    nc.sync.dma_start(
                        out=T[v][:, h * NS:h * NS + Wp],
                        in_=xp[rbase + v:rbase + v + 128, :])

            S0 = [psum.tile([128, W], F32, tag=f"S0p{h}", name=f"S0p{h}")
                  for h in range(nh)]
            S1 = [psum.tile([128, W], F32, tag=f"S1p{h}", name=f"S1p{h}")
                  for h in range(nh)]


            for oi, (di, dj) in enumerate(HALF_OFFSETS):
                cl = min(0, -dj)
                ch = max(W, W - dj)
                wd = ch - cl
                t_bias = (di + 2) * 5 + (dj + 2)

                dt = fld.tile([128, nh * NS], F16, tag="dt", name="dt")
                nc.vector.tensor_tensor(
                    out=pairap(dt[:, :], cl + 2, nh, wd),
                    in0=pairap(T[di][:, :], cl + dj + 2, nh, wd),
                    in1=pairap(T[0][:, :], cl + 2, nh, wd),
                    op=Alu.subtract)
                sq = fld.tile([128, nh * NS], F16, tag="sq", name="sq")
                if (di, dj) in SQ_ON_ACT:
                    nc.scalar.activation(
                        pairap(sq[:, :], cl + 2, nh, wd),
                        pairap(dt[:, :], cl + 2, nh, wd), Act.Square)
                else:
                    nc.vector.tensor_tensor(
                        out=pairap(sq[:, :], cl + 2, nh, wd),
                        in0=pairap(dt[:, :], cl + 2, nh, wd),
                        in1=pairap(dt[:, :], cl + 2, nh, wd),
                        op=Alu.mult)
                Wt = fld.tile([128, nh * NS], F16, tag="Wt", name="Wt")
                nc.scalar.activation(
                    pairap(Wt[:, :], cl + 2, nh, wd),
                    pairap(sq[:, :], cl + 2, nh, wd),
                    Act.Exp, bias=lt[:, t_bias:t_bias + 1], scale=EXP_SCALE)
                mt = fld.tile([128, nh * NS], F16, tag="mt", name="mt")
                nc.vector.tensor_tensor(
                    out=pairap(mt[:, :], cl + 2, nh, wd),
                    in0=pairap(Wt[:, :], cl + 2, nh, wd),
                    in1=pairap(dt[:, :], cl + 2, nh, wd),
                    op=Alu.mult)

                fw = 2          # tile col of output j=0, forward (j'+2)
                bw = 2 - dj     # backward
                last = oi == len(HALF_OFFSETS) - 1
                for h in range(nh):
                    o = h * NS
                    if dj == 0 and di in CMB:
                        c0, d0 = CMB[di]
                        nc.tensor.matmul(S0[h][:, :], c0[:, :],
                                         Wt[:, o + fw:o + fw + W],
                                         start=False, stop=last)
                        nc.tensor.matmul(S1[h][:, :], d0[:, :],
                                         mt[:, o + fw:o + fw + W],
                                         start=(oi == 0), stop=last)
                    else:
                        nc.tensor.matmul(S0[h][:, :], M[2][:, :],
                                         Wt[:, o + fw:o + fw + W],
                                         start=(oi == 0), stop=False)
                        nc.tensor.matmul(S0[h][:, :], M[2 - di][:, :],
                                         Wt[:, o + bw:o + bw + W],
                                         start=False, stop=last)
                        nc.tensor.matmul(S1[h][:, :], M[2][:, :],
                                         mt[:, o + fw:o + fw + W],
                                         start=(oi == 0), stop=False)
                        nc.tensor.matmul(S1[h][:, :], M[3 + 2 - di][:, :],
                                         mt[:, o + bw:o + bw + W],
                                         start=False, stop=last)

            for h, (rbase, K) in enumerate(grp):
                # evacuate PSUM via ScalarE (slack engine) so the banks free
                # for the next group's matmuls before the VectorE epilogue
                S0s = accp.tile([128, W], F32, tag="S0s", name="S0s")
                nc.scalar.activation(S0s[:K, :], S0[h][:K, :], Act.Identity,
                                     bias=sc[:K, :])
                S1s = accp.tile([128, W], F32, tag="S1s", name="S1s")
                nc.scalar.copy(S1s[:K, :], S1[h][:K, :])
                Rc = accp.tile([128, W], F32, tag="Rc", name="Rc")
                nc.vector.reciprocal_approx_fast(out=Rc[:K, :],
                                                 in_=S0s[:K, :])
                tmp = accp.tile([128, W], F32, tag="tmp", name="tmp")
                nc.gpsimd.tensor_tensor(
                    out=tmp[:K, :], in0=S1s[:K, :], in1=Rc[:K, :],
                    op=Alu.mult)
                res = accp.tile([128, W], F32, tag="res", name="res")
                nc.gpsimd.tensor_tensor(
                    out=res[:K, :], in0=tmp[:K, :],
                    in1=T[2][0:K, h * NS + 2:h * NS + 2 + W], op=Alu.add)

                k = 0
                while k < K:
                    g = rbase + 2 + k
                    if g < R and 2 <= (g % Hp) <= Hp - 3:
                        k1 = k
                        while k1 < K:
                            g1 = rbase + 2 + k1
                            if g1 >= R or not (2 <= (g1 % Hp) <= Hp - 3):
                                break
                            if (g1 % Hp) == 2 and k1 > k:
                                break
                            k1 += 1
                        h0 = (g // Hp) * H + (g % Hp) - 2
                        nc.sync.dma_start(out=y[h0:h0 + (k1 - k), :],
                                          in_=res[k:k1, :])
                        k = k1
                    else:
                        k += 1

    nc.compile()
    return nc


def _get_module():
    if "nc" not in _CACHE:
        _CACHE["nc"] = _build()
    return _CACHE["nc"]


def _pack_core(xc):
    """xc [C,H,W] f32 -> reflect-padded fp16 [Rpad, W+4]."""
    Rpad, _ = _strip_plan()
    xpad = np.pad(xc, ((0, 0), (2, 2), (2, 2)), mode="reflect")
    flat = xpad.reshape(C * (H + 4), W + 4)
    extra = Rpad - flat.shape[0]
    if extra > 0:
        flat = np.concatenate([flat, np.repeat(flat[-1:], extra, axis=0)],
                              axis=0)
    return np.ascontiguousarray(flat, dtype=np.float16)


def kernel(x, spatial, _trace=False):
    from concourse.bass_utils import run_bass_kernel_spmd

    x = np.asarray(x, dtype=np.float32)
    spatial = np.asarray(spatial, dtype=np.float32)
    assert x.shape == (B, C, H, W) and spatial.shape == (5, 5)
    # weight-field sharing between forward/backward taps needs symmetry
    assert np.allclose(spatial, spatial[::-1, ::-1], rtol=1e-5), \
        "kernel assumes point-symmetric spatial weights"

    lsp = np.log(spatial).reshape(1, 25).astype(np.float32)
    E = [np.eye(128, 128, k=-s, dtype=np.float32) for s in range(3)]
    mats = [e.astype(np.float16) for e in E]
    mats += [(-e).astype(np.float16) for e in E]
    mats.append(np.eye(128, dtype=np.float16)
                * np.float16(float(spatial[2, 2])))
    mats.append((E[2] + E[1]).astype(np.float16))  # C1: di=1 S0 combined
    mats.append((E[2] + E[0]).astype(np.float16))  # C0: di=2 S0 combined
    mats.append((E[2] - E[1]).astype(np.float16))  # D1: di=1 S1 combined
    mats.append((E[2] - E[0]).astype(np.float16))  # D0: di=2 S1 combined
    shm = np.concatenate(mats, axis=0)

    nc = _get_module()
    in_maps = [{"xp": _pack_core(x[b]), "lsp": lsp, "shm": shm}
               for b in range(B)]
    res = run_bass_kernel_spmd(nc, in_maps, core_ids=list(range(B)),
                               trace=_trace)
    out = np.stack([res.results[b]["y"].reshape(C, H, W) for b in range(B)])
    if _trace:
        return out.astype(np.float32), res
    return out.astype(np.float32)


# revision 8
# speedup vs baseline: 1.1229x; 1.0270x over previous
"""Bilateral filter denoiser (5x5, sigma_s=2.0, sigma_r=0.1) on 8 Trainium2
NeuronCores.  Takes full inputs x (8,3,512,512) f32 + spatial (5,5) f32;
pure data parallel: one batch element per core; returns the full output.

Per-core kernel (Bass/Tile), symmetric half-offset formulation:
  For each of the 12 half offsets t=(di,dj) the range/spatial weight field
      W_t[g] = exp(-50*(xp[g+t]-xp[g])^2 + ln s_t)
  is shared by the forward tap (pixel g gathering from g+t) and the backward
  tap (pixel g+t gathering from g).  With m_t = W_t*(xp[g+t]-xp[g]):
      S0 = s_c + sum_t (W_t[g] + W_t[g-t]),  S1 = sum_t (m_t[g] - m_t[g-t])
      out = x + S1/S0        (S0 >= s_c > 0: the 1e-10 clip never binds)
  Reflect padding makes the shared-weight trick exact at image borders.

Implementation notes:
  * Channels flattened along rows: xp [Rpad, 516] fp16 (host-padded).
    13 strips of 128 consecutive padded rows, processed in GROUPS OF 4:
    each field op (d/sq/exp/m) covers all 4 strips via a 2-level AP
    [[slot, 4], [1, wd]], amortizing the per-instruction fixed overhead
    (58cyc DVE / 224cyc ACT bubble) while keeping per-offset granularity
    so the engines pipeline offset-by-offset.
  * 3 row-shifted slab copies T0..T2 are DMA-loaded per group so every
    compute operand starts at partition 0 (HW rule: start partition must
    be 0/32/64/96); row shifts on computed fields instead run on the
    otherwise-idle TensorE as matmuls with host-provided +-shifted
    identity matrices, accumulating S0/S1 per strip in PSUM fp32
    (8 accumulators = all 8 PSUM banks).  dj==0 offsets use combined
    (M2 +- M_{2-di}) matrices: forward+backward in one matmul.
  * Fields are fp16 (DVE tensor ops at 2x); Square runs 6 offsets on
    ScalarE / 6 on VectorE (engine balance); Exp folds the -50 scale and
    ln(spatial) bias into ACT's free affine; s_center enters S0 via
    s_c*I @ ones.  Epilogue: custom-DVE fast reciprocal; result fp32.
"""

import numpy as np

B, C, H, W = 8, 3, 512, 512
SIGMA_R = 0.1
EXP_SCALE = -1.0 / (2.0 * SIGMA_R * SIGMA_R)  # -50

# order tuned for engine-queue interleave (di alternation): -4us vs row-major
HALF_OFFSETS = [
    (0, 1), (1, -2), (2, 2), (1, -1), (2, 1), (0, 2),
    (1, 0), (2, 0), (1, 1), (2, -1), (1, 2), (2, -2),
]
SQ_ON_ACT = {(0, 2), (1, -2), (1, 0), (1, 2), (2, -1)}
GSZ = 4

_CACHE = {}


def _strip_plan():
    Hp = H + 4
    R = C * Hp
    strips = []
    rbase = 0
    while R - 4 - rbase > 0:
        strips.append((rbase, min(124, R - 4 - rbase)))
        rbase += 124
    return strips[-1][0] + 132, strips


def _build():
    from contextlib import ExitStack

    import concourse.bacc as bacc
    import concourse.bass as bass
    import concourse.tile as tile
    from concourse import mybir

    F32 = mybir.dt.float32
    F16 = mybir.dt.float16
    Alu = mybir.AluOpType
    Act = mybir.ActivationFunctionType

    Hp, Wp = H + 4, W + 4
    R = C * Hp
    Rpad, strips = _strip_plan()
    NS = Wp  # per-strip slot width inside group tiles

    def pairap(v, col0, nh, wd):
        return bass.AP(tensor=v.tensor, offset=v.offset + col0,
                       ap=[v.ap[0], [NS, nh], [1, wd]])

    nc = bacc.Bacc(
        "TRN2",
        target_bir_lowering=False,
        debug=False,
        enable_asserts=False,
        num_devices=B,
    )
    xp = nc.dram_tensor("xp", [Rpad, Wp], F16, kind="ExternalInput").ap()
    lsp = nc.dram_tensor("lsp", [1, 25], F32, kind="ExternalInput").ap()
    shm = nc.dram_tensor("shm", [11 * 128, 128], F16, kind="ExternalInput").ap()
    y = nc.dram_tensor("y", [C * H, W], F32, kind="ExternalOutput").ap()

    with tile.TileContext(nc) as tc, ExitStack() as ctx:
        consts = ctx.enter_context(tc.tile_pool(name="consts", bufs=1))
        lt = consts.tile([128, 25], F32)
        nc.gpsimd.dma_start(out=lt[:], in_=lsp.to_broadcast([128, 25]))
        M = []  # [M0,M1,M2, N0,N1,N2, SC, C1,C0, D1,D0]
        for s in range(11):
            m_ = consts.tile([128, 128], F16, name=f"shm{s}", tag=f"shm{s}")
            nc.sync.dma_start(out=m_[:, :], in_=shm[s * 128:(s + 1) * 128, :])
            M.append(m_)
        CMB = {1: (M[7], M[9]), 2: (M[8], M[10])}
        ones = consts.tile([128, W], F16)
        nc.vector.memset(ones[:], 1.0)
        sc = consts.tile([128, 1], F32)
        nc.scalar.activation(sc[:], lt[:, 12:13], Act.Exp)

        slabs = ctx.enter_context(tc.tile_pool(name="slabs", bufs=2))
        fld = ctx.enter_context(tc.tile_pool(name="fld", bufs=8))
        accp = ctx.enter_context(tc.tile_pool(name="accum", bufs=3))
        psum = ctx.enter_context(tc.tile_pool(name="psum", bufs=1, space="PSUM"))

        groups = [strips[i:i + GSZ] for i in range(0, len(strips), GSZ)]
        for grp in groups:
            nh = len(grp)
            T = [slabs.tile([128, nh * NS], F16, tag=f"T{v}", name=f"T{v}")
                 for v in range(3)]
            for v in range(3):
                for h, (rbase, K) in enumerate(grp):
                    nc.sync.dma_start(
                        out=T[v][:, h * NS:h * NS + Wp],
                        in_=xp[rbase + v:rbase + v + 128, :])

            S0 = [psum.tile([128, W], F32, tag=f"S0p{h}", name=f"S0p{h}")
                  for h in range(nh)]
            S1 = [psum.tile([128, W], F32, tag=f"S1p{h}", name=f"S1p{h}")
                  for h in range(nh)]


            for oi, (di, dj) in enumerate(HALF_OFFSETS):
                cl = min(0, -dj)
                ch = max(W, W - dj)
                wd = ch - cl
                t_bias = (di + 2) * 5 + (dj + 2)

                dt = fld.tile([128, nh * NS], F16, tag="dt", name="dt")
                nc.vector.tensor_tensor(
                    out=pairap(dt[:, :], cl + 2, nh, wd),
                    in0=pairap(T[di][:, :], cl + dj + 2, nh, wd),
                    in1=pairap(T[0][:, :], cl + 2, nh, wd),
                    op=Alu.subtract)
                sq = fld.tile([128, nh * NS], F16, tag="sq", name="sq")
                if (di, dj) in SQ_ON_ACT:
                    nc.scalar.activation(
                        pairap(sq[:, :], cl + 2, nh, wd),
                        pairap(dt[:, :], cl + 2, nh, wd), Act.Square)
                else:
                    nc.vector.tensor_tensor(
                        out=pairap(sq[:, :], cl + 2, nh, wd),
                        in0=pairap(dt[:, :], cl + 2, nh, wd),
                        in1=pairap(dt[:, :], cl + 2, nh, wd),
                        op=Alu.mult)
                Wt = fld.tile([128, nh * NS], F16, tag="Wt", name="Wt")
                nc.scalar.activation(
                    pairap(Wt[:, :], cl + 2, nh, wd),
                    pairap(sq[:, :], cl + 2, nh, wd),
                    Act.Exp, bias=lt[:, t_bias:t_bias + 1], scale=EXP_SCALE)
                mt = fld.tile([128, nh * NS], F16, tag="mt", name="mt")
                nc.vector.tensor_tensor(
                    out=pairap(mt[:, :], cl + 2, nh, wd),
                    in0=pairap(Wt[:, :], cl + 2, nh, wd),
                    in1=pairap(dt[:, :], cl + 2, nh, wd),
                    op=Alu.mult)

                fw = 2          # tile col of output j=0, forward (j'+2)
                bw = 2 - dj     # backward
                last = oi == len(HALF_OFFSETS) - 1
                for h in range(nh):
                    o = h * NS
                    if dj == 0 and di in CMB:
                        c0, d0 = CMB[di]
                        nc.tensor.matmul(S0[h][:, :], c0[:, :],
                                         Wt[:, o + fw:o + fw + W],
                                         start=False, stop=last)
                        nc.tensor.matmul(S1[h][:, :], d0[:, :],
                                         mt[:, o + fw:o + fw + W],
                                         start=(oi == 0), stop=last)
                    else:
                        nc.tensor.matmul(S0[h][:, :], M[2][:, :],
                                         Wt[:, o + fw:o + fw + W],
                                         start=(oi == 0), stop=False)
                        nc.tensor.matmul(S0[h][:, :], M[2 - di][:, :],
                                         Wt[:, o + bw:o + bw + W],
                                         start=False, stop=last)
                        nc.tensor.matmul(S1[h][:, :], M[2][:, :],
                                         mt[:, o + fw:o + fw + W],
                                         start=(oi == 0), stop=False)
                        nc.tensor.matmul(S1[h][:, :], M[3 + 2 - di][:, :],
                                         mt[:, o + bw:o + bw + W],
                                         start=False, stop=last)

            for h, (rbase, K) in enumerate(grp):
                # evacuate PSUM via ScalarE (slack engine) so the banks free
                # for the next group's matmuls before the VectorE epilogue
                S0s = accp.tile([128, W], F32, tag="S0s", name="S0s")
                nc.scalar.activation(S0s[:K, :], S0[h][:K, :], Act.Identity,
                                     bias=sc[:K, :])
                S1s = accp.tile([128, W], F32, tag="S1s", name="S1s")
                nc.scalar.copy(S1s[:K, :], S1[h][:K, :])
                Rc = accp.tile([128, W], F32, tag="Rc", name="Rc")
                nc.vector.reciprocal_approx_fast(out=Rc[:K, :],
                                                 in_=S0s[:K, :])
                tmp = accp.tile([128, W], F32, tag="tmp", name="tmp")
                nc.gpsimd.tensor_tensor(
                    out=tmp[:K, :], in0=S1s[:K, :], in1=Rc[:K, :],
                    op=Alu.mult)
                res = accp.tile([128, W], F32, tag="res", name="res")
                nc.gpsimd.tensor_tensor(
                    out=res[:K, :], in0=tmp[:K, :],
                    in1=T[2][0:K, h * NS + 2:h * NS + 2 + W], op=Alu.add)

                k = 0
                while k < K:
                    g = rbase + 2 + k
                    if g < R and 2 <= (g % Hp) <= Hp - 3:
                        k1 = k
                        while k1 < K:
                            g1 = rbase + 2 + k1
                            if g1 >= R or not (2 <= (g1 % Hp) <= Hp - 3):
                                break
                            if (g1 % Hp) == 2 and k1 > k:
                                break
                            k1 += 1
                        h0 = (g // Hp) * H + (g % Hp) - 2
                        nc.sync.dma_start(out=y[h0:h0 + (k1 - k), :],
                                          in_=res[k:k1, :])
                        k = k1
                    else:
                        k += 1

    nc.compile()
    return nc


def _get_module():
    if "nc" not in _CACHE:
        _CACHE["nc"] = _build()
    return _CACHE["nc"]


def _pack_core(xc):
    """xc [C,H,W] f32 -> reflect-padded fp16 [Rpad, W+4]."""
    Rpad, _ = _strip_plan()
    xpad = np.pad(xc, ((0, 0), (2, 2), (2, 2)), mode="reflect")
    flat = xpad.reshape(C * (H + 4), W + 4)
    extra = Rpad - flat.shape[0]
    if extra > 0:
        flat = np.concatenate([flat, np.repeat(flat[-1:], extra, axis=0)],
                              axis=0)
    return np.ascontiguousarray(flat, dtype=np.float16)


def kernel(x, spatial, _trace=False):
    from concourse.bass_utils import run_bass_kernel_spmd

    x = np.asarray(x, dtype=np.float32)
    spatial = np.asarray(spatial, dtype=np.float32)
    assert x.shape == (B, C, H, W) and spatial.shape == (5, 5)
    # weight-field sharing between forward/backward taps needs symmetry
    assert np.allclose(spatial, spatial[::-1, ::-1], rtol=1e-5), \
        "kernel assumes point-symmetric spatial weights"

    lsp = np.log(spatial).reshape(1, 25).astype(np.float32)
    E = [np.eye(128, 128, k=-s, dtype=np.float32) for s in range(3)]
    mats = [e.astype(np.float16) for e in E]
    mats += [(-e).astype(np.float16) for e in E]
    mats.append(np.eye(128, dtype=np.float16)
                * np.float16(float(spatial[2, 2])))
    mats.append((E[2] + E[1]).astype(np.float16))  # C1: di=1 S0 combined
    mats.append((E[2] + E[0]).astype(np.float16))  # C0: di=2 S0 combined
    mats.append((E[2] - E[1]).astype(np.float16))  # D1: di=1 S1 combined
    mats.append((E[2] - E[0]).astype(np.float16))  # D0: di=2 S1 combined
    shm = np.concatenate(mats, axis=0)

    nc = _get_module()
    in_maps = [{"xp": _pack_core(x[b]), "lsp": lsp, "shm": shm}
               for b in range(B)]
    res = run_bass_kernel_spmd(nc, in_maps, core_ids=list(range(B)),
                               trace=_trace)
    out = np.stack([res.results[b]["y"].reshape(C, H, W) for b in range(B)])
    if _trace:
        return out.astype(np.float32), res
    return out.astype(np.float32)
